# revision 26
# baseline (speedup 1.0000x reference)
"""MoE (top-2 of 8 experts) Trainium2 Bass kernel, data-parallel over tokens.

Strategy: the 16384 tokens are sharded 2048/core across 8 NeuronCores.
Each core:
  R. routes its tokens (fp32 router matmul, top-2 via DVE max/max_index,
     renormalized gate weights via sigmoid of the logit gap),
  P. computes per-expert compacted positions with a PE prefix-sum over
     selection masks (fp16, exact for counts < 2048),
  E. per expert: scatters token ids into a DRAM gather list, indirect-DMA
     gathers the selected token rows (bf16), PE-transposes them, runs both
     expert matmuls in bf16 (gelu+bias fused on the scalar engine, b2
     folded in as a rank-1 matmul), stages y per expert in DRAM (bf16),
  F. re-gathers each token's two expert rows and combines them with the
     gate weights (natural token order, so no weight gather is needed).

All inter-phase DRAM dependencies are declared with add_dep_helper
(Tile only tracks SBUF/PSUM tiles).
"""

import sys

if "/opt/trn_rl_repo" not in sys.path:
    sys.path.insert(0, "/opt/trn_rl_repo")

import ml_dtypes
import numpy as np

import concourse.bass as bass
import concourse.mybir as mybir
import concourse.tile as tile
from concourse.bass import IndirectOffsetOnAxis
from concourse.bass_utils import run_bass_kernel_spmd
from concourse.masks import make_identity, make_upper_triangular

f32 = mybir.dt.float32
f16 = mybir.dt.float16
bf16 = mybir.dt.bfloat16
i32 = mybir.dt.int32
u32 = mybir.dt.uint32
Alu = mybir.AluOpType
Act = mybir.ActivationFunctionType

P = 128
N_CORES = 8
B, L, D, E = 4, 4096, 1024, 8
T = (B * L) // N_CORES      # tokens per core
NB = T // P                 # 128-token blocks per core
KD = D // P                 # contraction chunks
C = 640                     # per-(core, expert) token capacity
TC = C // P                 # gathered 128-token chunks per expert
OOB = 3000.0                # sentinel index (> any valid row) -> DMA-skipped


def _split_multi_waits(nc):
    """walrus here supports one semaphore wait per instruction; hoist
    extra waits onto single-wait NOPs just before the instruction."""
    ctr = 0
    for f in nc.m.functions:
        for bb in f.blocks:
            old = list(bb.instructions)
            new = []
            changed = False
            for inst in old:
                si = getattr(inst, "sync_info", None)
                waits = list(si.on_wait) if si is not None and si.on_wait else []
                if len(waits) > 1:
                    changed = True
                    for w in waits[:-1]:
                        ctr += 1
                        nop = mybir.InstNoOp(
                            name=f"I-waitsplit-{ctr}",
                            sync_info=mybir.SyncInfo(on_wait=[w], on_update=[]),
                            bass_nofuse=True,
                            engine=inst.engine,
                        )
                        nc.register_instruction(nop, overwrite=True)
                        new.append(nop)
                    del si.on_wait[:-1]
                new.append(inst)
            if changed:
                bb.instructions = new
    return ctr


def _build():
    nc = bass.Bass("TRN2", num_devices=N_CORES, num_swdge_queues=4)

    xT = nc.declare_dram_parameter("xT", [D, T], f32, isOutput=False)
    x_bf = nc.declare_dram_parameter("x_bf", [T, D], bf16, isOutput=False)
    wr = nc.declare_dram_parameter("wr", [D, E], f32, isOutput=False)
    w1t = nc.declare_dram_parameter("w1t", [E, D, D], bf16, isOutput=False)
    w2t = nc.declare_dram_parameter("w2t", [E, D, D], bf16, isOutput=False)
    b1d = nc.declare_dram_parameter("b1d", [P, E * KD], f32, isOutput=False)
    b2d = nc.declare_dram_parameter("b2d", [1, E * D], bf16, isOutput=False)
    out = nc.declare_dram_parameter("out", [T, D], f32, isOutput=True)

    y_all = nc.dram_tensor("y_all", [E * C, D], bf16)
    gall = nc.dram_tensor("gall", [E * C], f32)

    with tile.TileContext(nc) as tc:
        with tc.tile_pool(name="persist", bufs=1) as pp:
            # ---- constants ----
            ident_f32 = pp.tile([P, P], f32, tag="idf32")
            make_identity(nc, ident_f32[:])
            ident_bf = pp.tile([P, P], bf16, tag="idbf")
            make_identity(nc, ident_bf[:])
            ident_f16 = pp.tile([P, P], f16, tag="idf16")
            make_identity(nc, ident_f16[:])
            u128 = pp.tile([P, P], f16, tag="u128")
            make_upper_triangular(nc, u128[:], val=1.0, diag=True)
            u16s = pp.tile([16, 16], f16, tag="u16s")
            make_upper_triangular(nc, u16s[:], val=1.0, diag=False)
            ones_bf = pp.tile([1, P], bf16, tag="onesbf")
            nc.vector.memset(ones_bf[:], 1.0)

            iota_e_i = pp.tile([P, NB * E], i32, tag="iotaei")
            nc.gpsimd.iota(
                iota_e_i[:], pattern=[[0, NB], [1, E]], base=0, channel_multiplier=0
            )
            iota_e = pp.tile([P, NB * E], f32, tag="iotae")
            nc.vector.tensor_copy(out=iota_e[:], in_=iota_e_i[:])
            ebase_i = pp.tile([P, NB * E], i32, tag="ebasei")
            nc.gpsimd.iota(
                ebase_i[:], pattern=[[0, NB], [C, E]], base=0, channel_multiplier=0
            )
            ebase = pp.tile([P, NB * E], f32, tag="ebase")
            nc.vector.tensor_copy(out=ebase[:], in_=ebase_i[:])
            tokid_i = pp.tile([P, NB], i32, tag="tokidi")
            nc.gpsimd.iota(tokid_i[:], pattern=[[P, NB]], base=0, channel_multiplier=1)
            tokidf = pp.tile([P, NB], f32, tag="tokidf")
            nc.vector.tensor_copy(out=tokidf[:], in_=tokid_i[:])
            c_oob = pp.tile([P, E * TC], f32, tag="coob")
            nc.vector.memset(c_oob[:], OOB)

            b1_sb = pp.tile([P, E * KD], f32, tag="b1sb")
            nc.sync.dma_start(out=b1_sb[:], in_=b1d[:])
            b2_sb = pp.tile([1, E * D], bf16, tag="b2sb")
            nc.sync.dma_start(out=b2_sb[:], in_=b2d[:])

            # ---- persistent routing state ----
            mask_f16 = pp.tile([P, NB * E], f16, tag="maskf16")
            oh1_all = pp.tile([P, NB * E], f32, tag="oh1all")
            oh2_all = pp.tile([P, NB * E], f32, tag="oh2all")
            ps32 = pp.tile([E, NB * P], f32, tag="ps32")
            gl_i32 = pp.tile([P, E * TC], i32, tag="gli32")
            ptr0 = pp.tile([P, NB], i32, tag="ptr0")
            ptr1 = pp.tile([P, NB], i32, tag="ptr1")
            wt1_all = pp.tile([P, NB], f32, tag="wt1all")
            wt2_all = pp.tile([P, NB], f32, tag="wt2all")

            bc_tok = nc.gpsimd.to_reg(T - 1)
            bc_yall = nc.gpsimd.to_reg(E * C - 1)

            # sentinel-fill the gather list early (reads skip OOB rows)
            gall_fill = nc.sync.dma_start(
                out=gall.rearrange("(c p) -> p c", p=P), in_=c_oob[:]
            )

            # ================= PHASE R: router =================
            with (
                tc.tile_pool(name="rxt", bufs=1) as rxt,
                tc.tile_pool(name="rsb", bufs=2) as rsb,
                tc.tile_pool(name="rps", bufs=1, space="PSUM") as rps,
                tc.tile_pool(name="rtr", bufs=2, space="PSUM") as rtr,
            ):
                wr_sb = rxt.tile([P, KD * E], f32, tag="wrsb")
                nc.sync.dma_start(
                    out=wr_sb[:].rearrange("p (kd e) -> p kd e", kd=KD),
                    in_=wr.rearrange("(kd p) e -> p kd e", p=P),
                )
                xt_strips = []
                prev_dma = None
                for kd in range(KD):
                    strip = rxt.tile([P, T], f32, tag=f"xts{kd}")
                    dma = nc.sync.dma_start(
                        out=strip[:], in_=xT[kd * P : (kd + 1) * P, :]
                    )
                    if prev_dma is not None:
                        # serialize so strip 0 lands quickly and the router
                        # pipeline starts immediately
                        tile.add_dep_helper(
                            dma.ins, prev_dma.ins, sync=True, reason="strip chain"
                        )
                    prev_dma = dma
                    xt_strips.append(strip)
                psum_lt = rps.tile([E, T], f32, tag="psumlt")
                for kd in range(KD):
                    for j in range(T // 512):
                        nc.tensor.matmul(
                            out=psum_lt[:, j * 512 : (j + 1) * 512],
                            lhsT=wr_sb[:, kd * E : (kd + 1) * E],
                            rhs=xt_strips[kd][:, j * 512 : (j + 1) * 512],
                            start=(kd == 0),
                            stop=(kd == KD - 1),
                        )
                lt_sb = rxt.tile([E, T], f32, tag="ltsb")
                nc.vector.tensor_copy(out=lt_sb[:], in_=psum_lt[:])

                mx_all = rxt.tile([P, NB * E], f32, tag="mxall")
                ixu_all = rxt.tile([P, NB * E], u32, tag="ixuall")
                for tb in range(NB):
                    ptr_ps = rtr.tile([P, E], f32, tag="rtrp")
                    nc.tensor.transpose(
                        out=ptr_ps[:],
                        in_=lt_sb[:, tb * P : (tb + 1) * P],
                        identity=ident_f32[:E, :E],
                    )
                    lg = rsb.tile([P, E], f32, tag="lg")
                    nc.vector.tensor_copy(out=lg[:], in_=ptr_ps[:])
                    nc.vector.max(out=mx_all[:, tb * E : (tb + 1) * E], in_=lg[:])
                    nc.vector.max_index(
                        out=ixu_all[:, tb * E : (tb + 1) * E],
                        in_max=mx_all[:, tb * E : (tb + 1) * E],
                        in_values=lg[:],
                    )
                # batched gate weights: wt1 = sigmoid(l1 - l2), wt2 = 1 - wt1
                d12 = rsb.tile([P, NB], f32, tag="d12")
                nc.vector.tensor_tensor(
                    out=d12[:],
                    in0=mx_all[:].rearrange("p (t e) -> p t e", e=E)[:, :, 0],
                    in1=mx_all[:].rearrange("p (t e) -> p t e", e=E)[:, :, 1],
                    op=Alu.subtract,
                )
                nc.scalar.activation(wt1_all[:], d12[:], Act.Sigmoid)
                nc.scalar.activation(wt2_all[:], d12[:], Act.Sigmoid, scale=-1.0)
                # batched one-hots over all blocks
                ix1 = rsb.tile([P, NB], f32, tag="ix1")
                ix2 = rsb.tile([P, NB], f32, tag="ix2")
                nc.vector.tensor_copy(
                    out=ix1[:],
                    in_=ixu_all[:].rearrange("p (t e) -> p t e", e=E)[:, :, 0],
                )
                nc.vector.tensor_copy(
                    out=ix2[:],
                    in_=ixu_all[:].rearrange("p (t e) -> p t e", e=E)[:, :, 1],
                )
                nc.vector.tensor_tensor(
                    out=oh1_all[:],
                    in0=ix1[:, :, None].to_broadcast([P, NB, E]),
                    in1=iota_e[:].rearrange("p (t e) -> p t e", e=E),
                    op=Alu.is_equal,
                )
                nc.vector.tensor_tensor(
                    out=oh2_all[:],
                    in0=ix2[:, :, None].to_broadcast([P, NB, E]),
                    in1=iota_e[:].rearrange("p (t e) -> p t e", e=E),
                    op=Alu.is_equal,
                )
                msk = rsb.tile([P, NB * E], f32, tag="msk")
                nc.vector.tensor_add(msk[:], oh1_all[:], oh2_all[:])
                nc.vector.tensor_copy(out=mask_f16[:], in_=msk[:])

            # ================= PHASE P: prefix-sum positions =================
            # pass 1: per-block inclusive prefix counts (one U-matmul each)
            id_scat = []
            with (
                tc.tile_pool(name="pps", bufs=3, space="PSUM") as pps,
                tc.tile_pool(name="ptr2", bufs=3, space="PSUM") as ptr2,
                tc.tile_pool(name="psb", bufs=3) as psb,
            ):
                for tb in range(NB):
                    ps = pps.tile([E, P], f32, tag="ps")
                    nc.tensor.matmul(
                        out=ps[:],
                        lhsT=mask_f16[:, tb * E : (tb + 1) * E],
                        rhs=u128[:],
                        start=True,
                        stop=True,
                    )
                    nc.vector.tensor_copy(
                        out=ps32[:, tb * P : (tb + 1) * P], in_=ps[:]
                    )
                # cross-block exclusive offsets via a 16x16 strict-triangular mm
                tot16 = psb.tile([E, NB], f16, tag="tot16")
                nc.vector.tensor_copy(out=tot16[:], in_=ps32[:, P - 1 :: P])
                ptot = ptr2.tile([NB, E], f16, tag="pst")
                nc.tensor.transpose(
                    out=ptot[:], in_=tot16[:], identity=ident_f16[:E, :E]
                )
                totT = psb.tile([NB, E], f16, tag="totT")
                nc.vector.tensor_copy(out=totT[:], in_=ptot[:])
                poff = pps.tile([E, NB], f32, tag="ps")
                nc.tensor.matmul(
                    out=poff[:], lhsT=totT[:], rhs=u16s[:], start=True, stop=True
                )
                off_sb = psb.tile([E, NB], f32, tag="offsb")
                nc.vector.tensor_copy(out=off_sb[:], in_=poff[:])

                # pass 2: add block offsets, transpose to token-major positions
                pos_all = psb.tile([P, NB * E], f32, tag="posall")
                for tb in range(NB):
                    psg = psb.tile([E, P], f16, tag="psg")
                    nc.vector.tensor_scalar(
                        out=psg[:], in0=ps32[:, tb * P : (tb + 1) * P],
                        scalar1=off_sb[:, tb : tb + 1], scalar2=None, op0=Alu.add,
                    )
                    pst = ptr2.tile([P, E], f16, tag="pst")
                    nc.tensor.transpose(
                        out=pst[:], in_=psg[:], identity=ident_f16[:E, :E]
                    )
                    nc.vector.tensor_copy(
                        out=pos_all[:, tb * E : (tb + 1) * E], in_=pst[:]
                    )
                # batched pointers: ptrK[t] = sum_e ohK[t,e]*(pos[t,e]-1+640*e)
                pv = psb.tile([P, NB * E], f32, tag="pv")
                nc.vector.tensor_scalar(
                    out=pv[:], in0=pos_all[:], scalar1=-1.0, scalar2=None, op0=Alu.add
                )
                nc.vector.tensor_add(pv[:], pv[:], ebase[:])
                pt = psb.tile([P, NB * E], f32, tag="pt")
                prf = psb.tile([P, NB], f32, tag="prf")
                nc.vector.tensor_mul(pt[:], pv[:], oh1_all[:])
                nc.vector.tensor_reduce(
                    out=prf[:], in_=pt[:].rearrange("p (t e) -> p t e", e=E),
                    axis=mybir.AxisListType.X, op=Alu.add,
                )
                nc.vector.tensor_copy(out=ptr0[:], in_=prf[:])
                nc.vector.tensor_mul(pt[:], pv[:], oh2_all[:])
                nc.vector.tensor_reduce(
                    out=prf[:], in_=pt[:].rearrange("p (t e) -> p t e", e=E),
                    axis=mybir.AxisListType.X, op=Alu.add,
                )
                nc.vector.tensor_copy(out=ptr1[:], in_=prf[:])
                # burst-scatter all blocks' token ids into the gather list
                prev_scat = None
                for tb in range(NB):
                    for ptrcol in (ptr0, ptr1):
                        s = nc.gpsimd.indirect_dma_start(
                            out=gall[:, None],
                            out_offset=IndirectOffsetOnAxis(
                                ap=ptrcol[:, tb : tb + 1], axis=0
                            ),
                            in_=tokidf[:, tb : tb + 1],
                            in_offset=None,
                            bounds_check=bc_yall,
                            oob_is_err=False,
                        )
                        tile.add_dep_helper(
                            s.ins, gall_fill.ins, sync=True, reason="fill->scat"
                        )
                        if prev_scat is not None:
                            tile.add_dep_helper(
                                s.ins, prev_scat.ins, sync=False, reason="scat order"
                            )
                        prev_scat = s
                        id_scat.append(s)

            # ================= PHASE E: experts (with inline compaction) ====
            y_writes = []
            with (
                tc.tile_pool(name="ew", bufs=2) as ew,
                tc.tile_pool(name="exg", bufs=4) as exg,
                tc.tile_pool(name="ext", bufs=2) as ext,
                tc.tile_pool(name="eh", bufs=2) as eh,
                tc.tile_pool(name="ey", bufs=2) as ey,
                tc.tile_pool(name="eph", bufs=2, space="PSUM") as eph,
                tc.tile_pool(name="epy", bufs=2, space="PSUM") as epy,
                tc.tile_pool(name="ept", bufs=2, space="PSUM") as ept,
            ):
                for e in range(E):
                    # read back this expert's slice of the gather list
                    glf = ew.tile([P, TC], f32, tag="glf")
                    r = nc.sync.dma_start(
                        out=glf[:],
                        in_=gall[e * C : (e + 1) * C].rearrange("(c p) -> p c", p=P),
                    )
                    tile.add_dep_helper(
                        r.ins, id_scat[-1].ins, sync=True, reason="scat->rd"
                    )
                    nc.vector.tensor_copy(
                        out=gl_i32[:, e * TC : (e + 1) * TC], in_=glf[:]
                    )

                    w1_sb = ew.tile([P, KD * D], bf16, tag="w1sb")
                    nc.sync.dma_start(
                        out=w1_sb[:].rearrange("p (kd f) -> p kd f", kd=KD),
                        in_=w1t[e].rearrange("(kd p) f -> p kd f", p=P),
                    )
                    w2_sb = ew.tile([P, KD * D], bf16, tag="w2sb")
                    nc.sync.dma_start(
                        out=w2_sb[:].rearrange("p (fk d) -> p fk d", fk=KD),
                        in_=w2t[e].rearrange("(fk p) d -> p fk d", p=P),
                    )
                    # gather selected token rows and transpose to [d, t]
                    xgT = ext.tile([P, KD * C], bf16, tag="xgT")
                    for tcc in range(TC):
                        xg = exg.tile([P, D], bf16, tag="xg")
                        nc.gpsimd.indirect_dma_start(
                            out=xg[:],
                            out_offset=None,
                            in_=x_bf[:, :],
                            in_offset=IndirectOffsetOnAxis(
                                ap=gl_i32[:, e * TC + tcc : e * TC + tcc + 1], axis=0
                            ),
                            bounds_check=bc_tok,
                            oob_is_err=False,
                        )
                        for g in range(2):
                            tpt = ept.tile([P, 4 * P], bf16, tag="tpt")
                            for q in range(4):
                                kd = 4 * g + q
                                nc.tensor.transpose(
                                    out=tpt[:, q * P : (q + 1) * P],
                                    in_=xg[:, kd * P : (kd + 1) * P],
                                    identity=ident_bf[:],
                                )
                            nc.vector.tensor_copy(
                                out=xgT[:].rearrange("p (kd c) -> p kd c", kd=KD)[
                                    :, 4 * g : 4 * g + 4, tcc * P : (tcc + 1) * P
                                ],
                                in_=tpt[:].rearrange("p (q c) -> p q c", q=4),
                            )
                    # h.T = gelu(W1[e].T-chunks @ x-chunks + b1)
                    hT = eh.tile([P, KD * C], bf16, tag="hT")
                    for fc in range(KD):
                        ph = eph.tile([P, C], f32, tag="ph")
                        for kd in range(KD):
                            for n0, nl in ((0, 512), (512, C - 512)):
                                nc.tensor.matmul(
                                    out=ph[:, n0 : n0 + nl],
                                    lhsT=w1_sb[:, kd * D + fc * P : kd * D + (fc + 1) * P],
                                    rhs=xgT[:, kd * C + n0 : kd * C + n0 + nl],
                                    start=(kd == 0),
                                    stop=(kd == KD - 1),
                                )
                        nc.scalar.activation(
                            hT[:, fc * C : (fc + 1) * C],
                            ph[:],
                            Act.Gelu,
                            bias=b1_sb[:, e * KD + fc : e * KD + fc + 1],
                        )
                    # y = h @ W2[e].T + b2  (gate weight applied in phase F)
                    y_sb = ey.tile([P, TC * D], bf16, tag="ysb")
                    for tcc in range(TC):
                        for dc in range(2):
                            py = epy.tile([P, 512], f32, tag="py")
                            nc.tensor.matmul(
                                out=py[:],
                                lhsT=ones_bf[:],
                                rhs=b2_sb[0:1, e * D + dc * 512 : e * D + (dc + 1) * 512],
                                start=True,
                                stop=False,
                            )
                            for fc in range(KD):
                                nc.tensor.matmul(
                                    out=py[:],
                                    lhsT=hT[:, fc * C + tcc * P : fc * C + (tcc + 1) * P],
                                    rhs=w2_sb[:, fc * D + dc * 512 : fc * D + (dc + 1) * 512],
                                    start=False,
                                    stop=(fc == KD - 1),
                                )
                            nc.vector.tensor_copy(
                                out=y_sb[:, tcc * D + dc * 512 : tcc * D + (dc + 1) * 512],
                                in_=py[:],
                            )
                    y_writes.append(
                        nc.sync.dma_start(
                            out=y_all[e * C : (e + 1) * C, :].rearrange(
                                "(c p) d -> p c d", p=P
                            ),
                            in_=y_sb[:].rearrange("p (c d) -> p c d", c=TC),
                        )
                    )

            y_nop = nc.gpsimd.nop()
            for w in y_writes:
                tile.add_dep_helper(y_nop.ins, w.ins, sync=True, reason="y rdy")

            # ================= PHASE F: combine =================
            with (
                tc.tile_pool(name="fg", bufs=4) as fg,
                tc.tile_pool(name="fo", bufs=3) as fo,
            ):
                for tb in range(NB):
                    g0 = fg.tile([P, D], bf16, tag="g0")
                    i0 = nc.gpsimd.indirect_dma_start(
                        out=g0[:],
                        out_offset=None,
                        in_=y_all[:, :],
                        in_offset=IndirectOffsetOnAxis(
                            ap=ptr0[:, tb : tb + 1], axis=0
                        ),
                        bounds_check=bc_yall,
                        oob_is_err=False,
                    )
                    tile.add_dep_helper(i0.ins, y_nop.ins, sync=True, reason="y->g0")
                    g1 = fg.tile([P, D], bf16, tag="g1")
                    i1 = nc.gpsimd.indirect_dma_start(
                        out=g1[:],
                        out_offset=None,
                        in_=y_all[:, :],
                        in_offset=IndirectOffsetOnAxis(
                            ap=ptr1[:, tb : tb + 1], axis=0
                        ),
                        bounds_check=bc_yall,
                        oob_is_err=False,
                    )
                    tile.add_dep_helper(i1.ins, y_nop.ins, sync=True, reason="y->g1")
                    t0 = fo.tile([P, D], f32, tag="t0")
                    nc.vector.tensor_scalar(
                        out=t0[:], in0=g0[:], scalar1=wt1_all[:, tb : tb + 1],
                        scalar2=None, op0=Alu.mult,
                    )
                    t1 = fo.tile([P, D], f32, tag="t1")
                    nc.vector.tensor_scalar(
                        out=t1[:], in0=g1[:], scalar1=wt2_all[:, tb : tb + 1],
                        scalar2=None, op0=Alu.mult,
                    )
                    ob = fo.tile([P, D], f32, tag="ob")
                    nc.vector.tensor_add(ob[:], t0[:], t1[:])
                    nc.sync.dma_start(out=out[tb * P : (tb + 1) * P, :], in_=ob[:])

    _split_multi_waits(nc)
    return nc


_nc_cache = None


def kernel(x, Wr, W1, b1, W2, b2):
    global _nc_cache
    if _nc_cache is None:
        _nc_cache = _build()
    nc = _nc_cache

    x = np.asarray(x, dtype=np.float32)
    Wr = np.asarray(Wr, dtype=np.float32)
    W1 = np.asarray(W1, dtype=np.float32)
    b1 = np.asarray(b1, dtype=np.float32)
    W2 = np.asarray(W2, dtype=np.float32)
    b2 = np.asarray(b2, dtype=np.float32)

    xf = x.reshape(-1, D)
    wr_h = np.ascontiguousarray(Wr.T)
    w1t_h = np.ascontiguousarray(np.transpose(W1, (0, 2, 1))).astype(ml_dtypes.bfloat16)
    w2t_h = np.ascontiguousarray(np.transpose(W2, (0, 2, 1))).astype(ml_dtypes.bfloat16)
    b1d_h = np.ascontiguousarray(
        b1.reshape(E, KD, P).transpose(2, 0, 1).reshape(P, E * KD)
    )
    b2d_h = b2.reshape(1, E * D).astype(ml_dtypes.bfloat16)

    in_maps = []
    for i in range(N_CORES):
        s = slice(i * T, (i + 1) * T)
        in_maps.append(
            {
                "xT": np.ascontiguousarray(xf[s].T),
                "x_bf": xf[s].astype(ml_dtypes.bfloat16),
                "wr": wr_h,
                "w1t": w1t_h,
                "w2t": w2t_h,
                "b1d": b1d_h,
                "b2d": b2d_h,
            }
        )

    res = run_bass_kernel_spmd(nc, in_maps, core_ids=list(range(N_CORES)))
    out = np.concatenate(
        [res.results[i]["out"] for i in range(N_CORES)], axis=0
    ).reshape(B, L, D)
    return out
